# revision 1
# baseline (speedup 1.0000x reference)
"""Trainium2 Bass kernel for nn_LoopWithIf.

The reference loop
    for i in range(32):
        b = 3*a; s = sum(b); a = a+b if s>0 else a-b
collapses algebraically: sum(b) keeps its sign deterministic across
iterations (scaling by 4 or -2 is an exact power-of-two operation in
fp32), so the whole loop is a single scalar multiply:
    out = inp * 2**64      if sum(inp) > 0
    out = inp * -(2**63)   otherwise
Both factors are powers of two -> the multiply is exact in fp32.

Kernel structure (single NEFF, SPMD over 8 NeuronCores):
  - each core owns 4 of the 32 rows (16.8 MB), kept SBUF-resident
  - phase 1: pipelined DMA-in + per-chunk reduce_sum (DVE)
  - tiny AllReduce of per-partition partials across the 8 cores
  - ones-matmul broadcasts the global sum to all 128 partitions (PE)
  - factor = select(sum > 0, 2**64, -2**63) via two DVE tensor_scalar ops
  - phase 2: in-place scale per chunk (DVE) + pipelined DMA-out
"""

import numpy as np

N_CORES = 8
ROWS = 32            # inp.shape[0]
ROWS_PER_CORE = ROWS // N_CORES
P = 128              # SBUF partitions

# full tensor: (32, 1024, 1024) f32.  per-core shard: 4*1024*1024 = 4,194,304
# elements, reshaped [NCHUNK, P, F] chunk-major so each chunk is contiguous.
NCHUNK = 8
F = (ROWS_PER_CORE * 1024 * 1024) // (NCHUNK * P)   # 4096

_nc = None  # compiled kernel cache


def _build(nchunk=NCHUNK, p=P, f=F, n_cores=N_CORES):
    import concourse.bass as bass  # noqa: F401
    import concourse.bacc as bacc
    import concourse.mybir as mybir
    import concourse.tile as tile

    f32 = mybir.dt.float32
    nc = bacc.Bacc(
        "TRN2",
        target_bir_lowering=False,
        debug=False,
        enable_asserts=False,
        num_devices=n_cores,
    )
    inp_d = nc.dram_tensor("inp", [nchunk, p, f], f32, kind="ExternalInput").ap()
    out_d = nc.dram_tensor("out", [nchunk, p, f], f32, kind="ExternalOutput").ap()

    with tile.TileContext(nc) as tc:
        with (
            tc.tile_pool(name="data", bufs=1) as data_pool,
            tc.tile_pool(name="small", bufs=1) as small_pool,
            tc.tile_pool(name="psum", bufs=1, space="PSUM") as psum_pool,
            tc.tile_pool(name="dram", bufs=1, space="DRAM") as dram_pool,
        ):
            chunks = [
                data_pool.tile([p, f], f32, name=f"xchunk{i}", tag=f"xchunk{i}")
                for i in range(nchunk)
            ]
            partials = small_pool.tile([p, nchunk], f32, name="partials")
            ones = small_pool.tile([p, p], f32, name="ones")
            nc.vector.memset(ones[:], 1.0)

            # phase 1: load + per-chunk reduce
            for i in range(nchunk):
                nc.sync.dma_start(chunks[i][:], inp_d[i])
                nc.vector.reduce_sum(
                    partials[:, i : i + 1], chunks[i][:], axis=mybir.AxisListType.X
                )

            # local per-partition total
            plocal = small_pool.tile([p, 1], f32, name="plocal")
            nc.vector.reduce_sum(plocal[:], partials[:], axis=mybir.AxisListType.X)

            # cross-core sum of the [128,1] partials
            cc_in = dram_pool.tile([p, 1], f32, name="cc_in")
            cc_out = dram_pool.tile([p, 1], f32, name="cc_out", addr_space="Shared")
            nc.sync.dma_start(cc_in[:], plocal[:])
            nc.gpsimd.collective_compute(
                "AllReduce",
                mybir.AluOpType.add,
                replica_groups=[list(range(n_cores))],
                ins=[cc_in.opt()],
                outs=[cc_out.opt()],
            )
            q = small_pool.tile([p, 1], f32, name="q")
            nc.sync.dma_start(q[:], cc_out[:])

            # global total on every partition: ones[128,128].T @ q[128,1]
            tot = psum_pool.tile([p, 1], f32, name="tot")
            nc.tensor.matmul(tot[:], ones[:], q[:])

            # factor = 1[tot>0] * 3*2^63 - 2^63  ->  2^64 or -2^63 (exact)
            fac = small_pool.tile([p, 1], f32, name="fac")
            nc.vector.tensor_scalar(fac[:], tot[:], 0.0, None, mybir.AluOpType.is_gt)
            nc.vector.tensor_scalar(
                fac[:],
                fac[:],
                float(3 * 2**63),
                float(-(2**63)),
                mybir.AluOpType.mult,
                mybir.AluOpType.add,
            )

            # phase 2: in-place scale + store
            for i in range(nchunk):
                nc.vector.tensor_scalar_mul(chunks[i][:], chunks[i][:], fac[:])
                nc.sync.dma_start(out_d[i], chunks[i][:])

    nc.compile()
    return nc


def _run(in_maps, trace=False):
    from concourse.bass_utils import run_bass_kernel_spmd

    global _nc
    if _nc is None:
        _nc = _build()
    return run_bass_kernel_spmd(
        _nc, in_maps, core_ids=list(range(N_CORES)), trace=trace
    )


def _shard(inp):
    return [
        np.ascontiguousarray(
            inp[c * ROWS_PER_CORE : (c + 1) * ROWS_PER_CORE]
        ).reshape(NCHUNK, P, F)
        for c in range(N_CORES)
    ]


def _unshard(results):
    out = np.empty((ROWS, 1024, 1024), dtype=np.float32)
    for c in range(N_CORES):
        out[c * ROWS_PER_CORE : (c + 1) * ROWS_PER_CORE] = results[c]["out"].reshape(
            ROWS_PER_CORE, 1024, 1024
        )
    return out


def kernel(**inputs):
    inp = np.ascontiguousarray(np.asarray(inputs["inp"], dtype=np.float32))
    res = _run([{"inp": s} for s in _shard(inp)], trace=False)
    return _unshard(res.results)


def run_traced(inputs):
    """Like kernel() but with NTFF profiling; returns (out, exec_time_ns)."""
    inp = np.ascontiguousarray(np.asarray(inputs["inp"], dtype=np.float32))
    res = _run([{"inp": s} for s in _shard(inp)], trace=True)
    return _unshard(res.results), res.exec_time_ns
